# revision 16
# baseline (speedup 1.0000x reference)
"""Multi-head attention TRN2 Bass kernel (B=2, S=4096, D=256, H=8).

Sharding: 8 cores; core c handles batch c//4, a 1024-row query slice.
Each core computes its full output slice (all heads); host concatenates.

The exp bottleneck (ScalarE = 1 elem/cyc/lane, ~220us/core for the
33.5M score elements) is split 5/8 : 3/8 between the ScalarE ACT unit
and a DVE Schraudolph bit-trick exp:
  - wq is host-folded with A10 = 1024*log2(e)/sqrt(32), so scores psum
    holds t = 1024*log2(e)*y where y = qk/sqrt(dh) (f32r matmuls,
    4 heads row-tiled via 32-row tile_position strips, 2 concurrent).
  - One fp8e5 mask tensor m01 {1=keep, 0=masked} serves both paths.
  - ACT-share k-blocks: an fp8e5 DoubleRow ident matmul (diag 40960)
    adds +40960*m01 into psum; ACT computes exp(psum/A10R - 27.73)
    -> fp16 probs (masked rows die via the bias, unmasked exact).
  - DVE-share k-blocks: one scalar_tensor_tensor computes
    int16((psum + 15315.35) * m01); those int16 bits ARE fp16(e^y)
    (Schraudolph, ~±3% sawtooth); masked probs are exactly 0.
  - attn^T accumulates per head-pair in col-tiled (0,0)/(0,64) fp16
    matmuls over ones-augmented V (row 32 = softmax denominator).
  - K/V projections are emitted just-in-time inside the first q-chunk's
    k-loop so they overlap the main pipeline.
Engine budget (real-HW model): PE ~171us (with tile-position
concurrency), ACT ~160us, DVE ~167us vs the baseline's ACT-bound
~250us. TimelineSim reports 391us but serializes tile-position-
concurrent matmuls (it charges the baseline 351us vs its real 251us).
"""

import math
import os
import sys
import time

import numpy as np

sys.path.insert(0, "/opt/trn_rl_repo")

import ml_dtypes  # noqa: E402

import concourse.bass as bass  # noqa: E402
import concourse.mybir as mybir  # noqa: E402
from concourse import bacc  # noqa: E402
from concourse.bass import ts  # noqa: E402
from concourse.bass_utils import run_bass_kernel_spmd  # noqa: E402
from concourse.tile import TileContext  # noqa: E402

B = 2
S = 4096
D = 256
H = 8
DH = 32
NCORES = 8
CORES_PER_B = 4
QS = S // CORES_PER_B  # 1024 query rows per core
QCHUNK = 512
NQC = QS // QCHUNK  # 2
KB = S // 128  # 32 k-blocks
SCALE = 1.0 / math.sqrt(DH)
A10R = 1024.0 * math.log2(math.e)  # Schraudolph fp16 exponent multiplier
A10 = A10R * SCALE  # folded into wq on host: psum = A10R * (SCALE*qk)
BC = 15360.0 - 44.65  # Schraudolph magic (15<<10) - c
IDENT_DIAG = 40960.0  # fp8e5-exact; /A10R = 27.73 exp units (mask kill)
ACT_OF_8 = 5  # kb % 8 < ACT_OF_8 -> ACT share (a = 5/8)

F32 = mybir.dt.float32
F32R = mybir.dt.float32r
BF16 = mybir.dt.bfloat16
F16 = mybir.dt.float16
FP8E5 = mybir.dt.float8e5
I16 = mybir.dt.int16

LAST_EXEC_NS = None


def build_nc(act_of_8=ACT_OF_8):
    nc = bacc.Bacc(None)

    qT_d = nc.declare_dram_parameter("qT", [D, QS], F32R, isOutput=False)
    kT_d = nc.declare_dram_parameter("kT", [D, S], F32R, isOutput=False)
    vT_d = nc.declare_dram_parameter("vT", [D, S], F32R, isOutput=False)
    maskP_d = nc.declare_dram_parameter("maskP", [S, QS], FP8E5, isOutput=False)
    wq_d = nc.declare_dram_parameter("wq", [D, D], F32R, isOutput=False)
    wk_d = nc.declare_dram_parameter("wk", [D, D], F32R, isOutput=False)
    wv_d = nc.declare_dram_parameter("wv", [D, D], F32R, isOutput=False)
    wo_d = nc.declare_dram_parameter("wo", [D, D], F32R, isOutput=False)
    identdr_d = nc.declare_dram_parameter("identdr", [128, 2, 128], FP8E5, isOutput=False)
    out_d = nc.declare_dram_parameter("out", [QS, D], F32, isOutput=True)

    with TileContext(nc) as tc:
        with (
            tc.tile_pool(name="consts", bufs=1) as consts,
            tc.tile_pool(name="big", bufs=1) as big,
            tc.tile_pool(name="stream", bufs=6) as stream,
            tc.tile_pool(name="mpool", bufs=8) as mpool,
            tc.tile_pool(name="ppool", bufs=10) as ppool,
            tc.tile_pool(name="small", bufs=6) as small,
            tc.tile_pool(name="rpool", bufs=6) as rpool,
            tc.tile_pool(name="ostage", bufs=3) as ostage,
            tc.tile_pool(name="ps_sc", bufs=2, space="PSUM") as ps_sc,
            tc.tile_pool(name="ps_at", bufs=2, space="PSUM") as ps_at,
        ):
            # ---- constants ----
            identdr = consts.tile([128, 2, 128], FP8E5)
            nc.sync.dma_start(out=identdr, in_=identdr_d[:, :, :])
            wq_sb = consts.tile([128, 2, D], F32R)
            wk_sb = consts.tile([128, 2, D], F32R)
            wv_sb = consts.tile([128, 2, D], F32R)
            wo_sb = consts.tile([128, 2, D], F32R)
            for w_sb, w_d in ((wq_sb, wq_d), (wk_sb, wk_d), (wv_sb, wv_d), (wo_sb, wo_d)):
                nc.sync.dma_start(
                    out=w_sb, in_=w_d[:, :].rearrange("(c p) n -> p c n", p=128)
                )
            ones_src = consts.tile([128, KB * H], F16)
            nc.vector.memset(ones_src, 1.0)
            bias_t = consts.tile([128, 1], F32)
            nc.vector.memset(bias_t, -(IDENT_DIAG / A10R))

            # ---- persistent activations ----
            QT_sb = big.tile([128, 2, QS], F32R, name="QT_sb")
            KT_sb = big.tile([128, 2, S], F32R, name="KT_sb")
            V_sb = big.tile([128, KB, H, DH + 1], F16, name="V_sb")
            attnT_sb = big.tile([128, 2, QS], F32R, name="attnT_sb")
            nc.vector.tensor_copy(
                V_sb[:, :, :, DH], ones_src.rearrange("p (a b) -> p a b", a=KB)
            )

            # ---- projection chunk emitters (interleaved into qc=0 loop) ----
            def proj_chunk(src_d, dst, w_sb, c, csz=512):
                t0 = stream.tile([128, csz], F32R, tag="st")
                t1 = stream.tile([128, csz], F32R, tag="st")
                nc.sync.dma_start(out=t0, in_=src_d[0:128, ts(c, csz)])
                nc.sync.dma_start(out=t1, in_=src_d[128:256, ts(c, csz)])
                for half in range(2):
                    ps = ps_sc.tile([128, 2, csz], F32, tag="sc", name="psproj")
                    nc.tensor.matmul(
                        ps[:, 0, :], w_sb[:, 0, ts(half, 128)], t0,
                        start=True, stop=False,
                    )
                    nc.tensor.matmul(
                        ps[:, 0, :], w_sb[:, 1, ts(half, 128)], t1,
                        start=False, stop=True,
                    )
                    nc.vector.tensor_copy(dst[:, half, ts(c, csz)], ps[:, 0, :])

            def v_chunk(c):
                v0 = stream.tile([128, 512], F32R, tag="st")
                v1 = stream.tile([128, 512], F32R, tag="st")
                nc.sync.dma_start(out=v0, in_=vT_d[0:128, ts(c, 512)])
                nc.sync.dma_start(out=v1, in_=vT_d[128:256, ts(c, 512)])
                for sb_i in range(4):
                    kb = c * 4 + sb_i
                    pv = ps_sc.tile([128, 2, 512], F32, tag="sc", name="psv")
                    nc.tensor.matmul(
                        pv[:, 0, 0:D], v0[:, ts(sb_i, 128)], wv_sb[:, 0, :],
                        start=True, stop=False,
                    )
                    nc.tensor.matmul(
                        pv[:, 0, 0:D], v1[:, ts(sb_i, 128)], wv_sb[:, 1, :],
                        start=False, stop=True,
                    )
                    nc.vector.tensor_copy(
                        V_sb[:, kb, :, 0:DH],
                        pv[:, 0, 0:D].rearrange("p (h d) -> p h d", h=H),
                    )

            # QT fully upfront; KT/V chunks 0 upfront, rest just-in-time
            for c in range(QS // 512):
                proj_chunk(qT_d, QT_sb, wq_sb, c)
            proj_chunk(kT_d, KT_sb, wk_sb, 0)
            v_chunk(0)

            # ---- main attention loops ----
            for qc in range(NQC):
                at_tiles = [
                    ps_at.tile([128, 2, QCHUNK], F32, tag="at", name=f"at{j}")
                    for j in range(2)
                ]

                def at_ap(pr):
                    return at_tiles[pr // 2][:, pr % 2, :]

                for kb in range(KB):
                    if qc == 0 and kb % 4 == 0 and kb // 4 + 1 < KB // 4:
                        c = kb // 4 + 1
                        proj_chunk(kT_d, KT_sb, wk_sb, c)
                        v_chunk(c)
                    mt = mpool.tile([128, QCHUNK], FP8E5, tag="mt", name="mt")
                    nc.sync.dma_start(
                        out=mt, in_=maskP_d[ts(kb, 128), ts(qc, QCHUNK)]
                    )
                    mrhs = bass.AP(
                        tensor=mt.tensor,
                        offset=mt.offset,
                        ap=[mt.ap[0], [0, 2], mt.ap[1]],
                    )
                    pbs = []
                    for wave in range(4):
                        # Wave-granular ACT/DVE split (act_of_8 eighths per
                        # 2-kb period) so both engines stay fed every kb.
                        e = (kb % 2) * 4 + wave
                        is_act = ((0, 1, 3, 4, 6, 2, 5, 7).index(e) < act_of_8)
                        half = wave // 2
                        sc = ps_sc.tile(
                            [128, 2, QCHUNK], F32, tag="sc", name="sc"
                        )
                        for i in range(2):
                            h = wave * 2 + i
                            strip = (h % 4) * 32
                            nc.tensor.matmul(
                                sc[:, i, :],
                                KT_sb[strip : strip + 32, half, ts(kb, 128)],
                                QT_sb[strip : strip + 32, half, ts(qc, QCHUNK)],
                                start=True,
                                stop=(not is_act),
                                tile_position=(strip, 0),
                            )
                        pb = ppool.tile([128, 2, QCHUNK], F16, tag="pb", name="pb")
                        if is_act:
                            for i in range(2):
                                nc.tensor.matmul(
                                    sc[:, i, :],
                                    identdr,
                                    mrhs,
                                    start=False,
                                    stop=True,
                                    perf_mode=mybir.MatmulPerfMode.DoubleRow,
                                )
                            nc.scalar.activation(
                                pb[:, :, :],
                                sc[:, :, :],
                                mybir.ActivationFunctionType.Exp,
                                scale=1.0 / A10R,
                                bias=bias_t[:, 0:1],
                            )
                        else:
                            nc.vector.scalar_tensor_tensor(
                                out=pb.bitcast(I16),
                                in0=sc[:, :, :],
                                scalar=BC,
                                in1=mrhs,
                                op0=mybir.AluOpType.add,
                                op1=mybir.AluOpType.mult,
                            )
                        pbs.append(pb)
                    for wave in range(4):
                        pb = pbs[wave]
                        h0 = wave * 2
                        nc.tensor.matmul(
                            at_ap(wave)[0 : DH + 1, :],
                            V_sb[:, kb, h0, :],
                            pb[:, 0, :],
                            start=(kb == 0),
                            stop=(kb == KB - 1),
                            tile_position=(0, 0),
                        )
                        nc.tensor.matmul(
                            at_ap(wave)[64 : 64 + DH + 1, :],
                            V_sb[:, kb, h0 + 1, :],
                            pb[:, 1, :],
                            start=(kb == 0),
                            stop=(kb == KB - 1),
                            tile_position=(0, 64),
                        )

                # normalize: attnT = attn_unnorm^T * (1/denom) broadcast
                for j in range(2):
                    for i in range(2):
                        base = i * 64
                        rec = rpool.tile([1, 2, QCHUNK], F32, tag="rc", name="rec")
                        nc.vector.reciprocal(
                            rec, at_tiles[j][base + DH : base + DH + 1, :, :]
                        )
                        for pr in range(2):
                            w = j * 2 + pr
                            h = w * 2 + i
                            rb = small.tile([DH, QCHUNK], F32, tag="rb", name="rb")
                            nc.gpsimd.partition_broadcast(rb, rec[0:1, pr, :])
                            nc.vector.tensor_mul(
                                attnT_sb[ts(h % 4, DH), h // 4, ts(qc, QCHUNK)],
                                at_tiles[j][base : base + DH, pr, :],
                                rb,
                            )

                # output projection for this q-chunk
                for qb in range(QCHUNK // 128):
                    qoff = qc * QCHUNK + qb * 128
                    po = ps_sc.tile([128, 2, 512], F32, tag="sc", name="po")
                    nc.tensor.matmul(
                        po[:, 0, 0:D],
                        attnT_sb[:, 0, qoff : qoff + 128],
                        wo_sb[:, 0, :],
                        start=True,
                        stop=False,
                    )
                    nc.tensor.matmul(
                        po[:, 0, 0:D],
                        attnT_sb[:, 1, qoff : qoff + 128],
                        wo_sb[:, 1, :],
                        start=False,
                        stop=True,
                    )
                    ot = ostage.tile([128, D], F32, tag="ot", name="ot")
                    nc.scalar.copy(ot, po[:, 0, 0:D])
                    nc.sync.dma_start(out=out_d[qoff : qoff + 128, :], in_=ot)

    nc.finalize()
    return nc


_NC_CACHE = None


def _get_nc():
    global _NC_CACHE
    if _NC_CACHE is None:
        _NC_CACHE = build_nc(int(os.environ.get("K_ACT_OF_8", str(ACT_OF_8))))
    return _NC_CACHE


def _prep_in_maps(q, k, v, mask, wq, wk, wv, w_out):
    f32 = np.float32
    bf16 = ml_dtypes.bfloat16
    qT = np.ascontiguousarray(np.transpose(np.asarray(q, f32), (0, 2, 1)))
    kT = np.ascontiguousarray(np.transpose(np.asarray(k, f32), (0, 2, 1)))
    vT = np.ascontiguousarray(np.transpose(np.asarray(v, f32), (0, 2, 1)))
    fp8e5 = ml_dtypes.float8_e5m2
    maskT = np.transpose(np.asarray(mask, bool), (0, 2, 1))
    maskP = (~maskT).astype(f32).astype(fp8e5)
    identdr = np.zeros((128, 2, 128), fp8e5)
    identdr[:, 0, :] = np.eye(128, dtype=f32) * f32(IDENT_DIAG)
    wq = np.asarray(wq, f32) * f32(A10)
    wk = np.asarray(wk, f32)
    wv = np.asarray(wv, f32)
    wo = np.asarray(w_out, f32)

    in_maps = []
    for c in range(NCORES):
        b = c // CORES_PER_B
        qs = slice((c % CORES_PER_B) * QS, (c % CORES_PER_B + 1) * QS)
        in_maps.append(
            {
                "qT": np.ascontiguousarray(qT[b][:, qs]),
                "kT": kT[b],
                "vT": vT[b],
                "maskP": np.ascontiguousarray(maskP[b][:, qs]),
                "wq": wq,
                "wk": wk,
                "wv": wv,
                "wo": wo,
                "identdr": identdr,
            }
        )
    return in_maps


def kernel(q, k, v, mask, wq, wk, wv, w_out):
    global LAST_EXEC_NS
    nc = _get_nc()
    in_maps = _prep_in_maps(q, k, v, mask, wq, wk, wv, w_out)
    trace = bool(os.environ.get("KERNEL_TRACE"))
    try:
        res = run_bass_kernel_spmd(nc, in_maps, list(range(NCORES)), trace=trace)
    except Exception:
        # A wedged NeuronCore (NRT_EXEC_UNIT_UNRECOVERABLE) is usually
        # transient under axon; one retry after a reset request recovers it.
        os.environ["NEURON_RT_RESET_CORES"] = "1"
        time.sleep(2)
        res = run_bass_kernel_spmd(nc, in_maps, list(range(NCORES)), trace=trace)
    LAST_EXEC_NS = res.exec_time_ns
    out = np.empty((B, S, D), np.float32)
    for c in range(NCORES):
        b = c // CORES_PER_B
        qs = slice((c % CORES_PER_B) * QS, (c % CORES_PER_B + 1) * QS)
        out[b, qs] = res.results[c]["out"]
    return out


# revision 18
# speedup vs baseline: 1.0714x; 1.0714x over previous
"""Multi-head attention TRN2 Bass kernel (B=2, S=4096, D=256, H=8).

Sharding: 8 cores; core c handles batch c//4, a 1024-row query slice.
Each core computes its full output slice (all heads); host concatenates.

The exp bottleneck (ScalarE = 1 elem/cyc/lane, ~220us/core for the
33.5M score elements) is split 5/8 : 3/8 between the ScalarE ACT unit
and a DVE Schraudolph bit-trick exp:
  - wq is host-folded with A10 = 1024*log2(e)/sqrt(32), so scores psum
    holds t = 1024*log2(e)*y where y = qk/sqrt(dh) (f32r matmuls,
    4 heads row-tiled via 32-row tile_position strips, 2 concurrent).
  - One fp8e5 mask tensor m01 {1=keep, 0=masked} serves both paths.
  - ACT-share k-blocks: an fp8e5 DoubleRow ident matmul (diag 40960)
    adds +40960*m01 into psum; ACT computes exp(psum/A10R - 27.73)
    -> fp16 probs (masked rows die via the bias, unmasked exact).
  - DVE-share k-blocks: one scalar_tensor_tensor computes
    int16((psum + 15315.35) * m01); those int16 bits ARE fp16(e^y)
    (Schraudolph, ~±3% sawtooth); masked probs are exactly 0.
  - attn^T accumulates per head-pair in col-tiled (0,0)/(0,64) fp16
    matmuls over ones-augmented V (row 32 = softmax denominator).
  - K/V projections are emitted just-in-time inside the first q-chunk's
    k-loop so they overlap the main pipeline.
  - The ACT/DVE split interleaves at WAVE granularity (per kb-pair:
    A,A,D,A | A,D,A,D) so both exp engines have work every iteration;
    ACT runs never exceed the 2-deep scores-psum buffering, so the DVE
    share never stalls dry (a kb-block split would serialize the two
    engines' duty cycles at ~274us).
Engine budget (real-HW model): PE ~171us (with tile-position
concurrency), ACT ~162us, DVE ~163us vs the baseline's ACT-bound
~250us. TimelineSim reports ~400us but serializes tile-position-
concurrent matmuls (it charges the baseline 351us vs its real 251us).
"""

import math
import os
import sys
import time

import numpy as np

sys.path.insert(0, "/opt/trn_rl_repo")

import ml_dtypes  # noqa: E402

import concourse.bass as bass  # noqa: E402
import concourse.mybir as mybir  # noqa: E402
from concourse import bacc  # noqa: E402
from concourse.bass import ts  # noqa: E402
from concourse.bass_utils import run_bass_kernel_spmd  # noqa: E402
from concourse.tile import TileContext  # noqa: E402

B = 2
S = 4096
D = 256
H = 8
DH = 32
NCORES = 8
CORES_PER_B = 4
QS = S // CORES_PER_B  # 1024 query rows per core
QCHUNK = 512
NQC = QS // QCHUNK  # 2
KB = S // 128  # 32 k-blocks
SCALE = 1.0 / math.sqrt(DH)
A10R = 1024.0 * math.log2(math.e)  # Schraudolph fp16 exponent multiplier
A10 = A10R * SCALE  # folded into wq on host: psum = A10R * (SCALE*qk)
BC = 15360.0 - 44.65  # Schraudolph magic (15<<10) - c
IDENT_DIAG = 40960.0  # fp8e5-exact; /A10R = 27.73 exp units (mask kill)
ACT_OF_8 = 5  # kb % 8 < ACT_OF_8 -> ACT share (a = 5/8)

F32 = mybir.dt.float32
F32R = mybir.dt.float32r
BF16 = mybir.dt.bfloat16
F16 = mybir.dt.float16
FP8E5 = mybir.dt.float8e5
I16 = mybir.dt.int16

LAST_EXEC_NS = None


def build_nc(act_of_8=ACT_OF_8):
    nc = bacc.Bacc(None)

    qT_d = nc.declare_dram_parameter("qT", [D, QS], F32R, isOutput=False)
    kT_d = nc.declare_dram_parameter("kT", [D, S], F32R, isOutput=False)
    vT_d = nc.declare_dram_parameter("vT", [D, S], F32R, isOutput=False)
    maskP_d = nc.declare_dram_parameter("maskP", [S, QS], FP8E5, isOutput=False)
    wq_d = nc.declare_dram_parameter("wq", [D, D], F32R, isOutput=False)
    wk_d = nc.declare_dram_parameter("wk", [D, D], F32R, isOutput=False)
    wv_d = nc.declare_dram_parameter("wv", [D, D], F32R, isOutput=False)
    wo_d = nc.declare_dram_parameter("wo", [D, D], F32R, isOutput=False)
    identdr_d = nc.declare_dram_parameter("identdr", [128, 2, 128], FP8E5, isOutput=False)
    out_d = nc.declare_dram_parameter("out", [QS, D], F32, isOutput=True)

    with TileContext(nc) as tc:
        with (
            tc.tile_pool(name="consts", bufs=1) as consts,
            tc.tile_pool(name="big", bufs=1) as big,
            tc.tile_pool(name="stream", bufs=6) as stream,
            tc.tile_pool(name="mpool", bufs=8) as mpool,
            tc.tile_pool(name="ppool", bufs=10) as ppool,
            tc.tile_pool(name="small", bufs=6) as small,
            tc.tile_pool(name="rpool", bufs=6) as rpool,
            tc.tile_pool(name="ostage", bufs=3) as ostage,
            tc.tile_pool(name="ps_sc", bufs=2, space="PSUM") as ps_sc,
            tc.tile_pool(name="ps_at", bufs=2, space="PSUM") as ps_at,
        ):
            # ---- constants ----
            identdr = consts.tile([128, 2, 128], FP8E5)
            nc.sync.dma_start(out=identdr, in_=identdr_d[:, :, :])
            wq_sb = consts.tile([128, 2, D], F32R)
            wk_sb = consts.tile([128, 2, D], F32R)
            wv_sb = consts.tile([128, 2, D], F32R)
            wo_sb = consts.tile([128, 2, D], F32R)
            for w_sb, w_d in ((wq_sb, wq_d), (wk_sb, wk_d), (wv_sb, wv_d), (wo_sb, wo_d)):
                nc.sync.dma_start(
                    out=w_sb, in_=w_d[:, :].rearrange("(c p) n -> p c n", p=128)
                )
            ones_src = consts.tile([128, KB * H], F16)
            nc.vector.memset(ones_src, 1.0)
            bias_t = consts.tile([128, 1], F32)
            nc.vector.memset(bias_t, -(IDENT_DIAG / A10R))

            # ---- persistent activations ----
            QT_sb = big.tile([128, 2, QS], F32R, name="QT_sb")
            KT_sb = big.tile([128, 2, S], F32R, name="KT_sb")
            V_sb = big.tile([128, KB, H, DH + 1], F16, name="V_sb")
            attnT_sb = big.tile([128, 2, QS], F32R, name="attnT_sb")
            nc.vector.tensor_copy(
                V_sb[:, :, :, DH], ones_src.rearrange("p (a b) -> p a b", a=KB)
            )

            # ---- projection chunk emitters (interleaved into qc=0 loop) ----
            def proj_chunk(src_d, dst, w_sb, c, csz=512):
                t0 = stream.tile([128, csz], F32R, tag="st")
                t1 = stream.tile([128, csz], F32R, tag="st")
                nc.sync.dma_start(out=t0, in_=src_d[0:128, ts(c, csz)])
                nc.sync.dma_start(out=t1, in_=src_d[128:256, ts(c, csz)])
                for half in range(2):
                    ps = ps_sc.tile([128, 2, csz], F32, tag="sc", name="psproj")
                    nc.tensor.matmul(
                        ps[:, 0, :], w_sb[:, 0, ts(half, 128)], t0,
                        start=True, stop=False,
                    )
                    nc.tensor.matmul(
                        ps[:, 0, :], w_sb[:, 1, ts(half, 128)], t1,
                        start=False, stop=True,
                    )
                    nc.vector.tensor_copy(dst[:, half, ts(c, csz)], ps[:, 0, :])

            def v_chunk(c):
                v0 = stream.tile([128, 512], F32R, tag="st")
                v1 = stream.tile([128, 512], F32R, tag="st")
                nc.sync.dma_start(out=v0, in_=vT_d[0:128, ts(c, 512)])
                nc.sync.dma_start(out=v1, in_=vT_d[128:256, ts(c, 512)])
                for sb_i in range(4):
                    kb = c * 4 + sb_i
                    pv = ps_sc.tile([128, 2, 512], F32, tag="sc", name="psv")
                    nc.tensor.matmul(
                        pv[:, 0, 0:D], v0[:, ts(sb_i, 128)], wv_sb[:, 0, :],
                        start=True, stop=False,
                    )
                    nc.tensor.matmul(
                        pv[:, 0, 0:D], v1[:, ts(sb_i, 128)], wv_sb[:, 1, :],
                        start=False, stop=True,
                    )
                    nc.vector.tensor_copy(
                        V_sb[:, kb, :, 0:DH],
                        pv[:, 0, 0:D].rearrange("p (h d) -> p h d", h=H),
                    )

            # QT chunk 0 + KT/V chunks 0 upfront, rest just-in-time
            proj_chunk(qT_d, QT_sb, wq_sb, 0)
            proj_chunk(kT_d, KT_sb, wk_sb, 0)
            v_chunk(0)

            # ---- main attention loops ----
            for qc in range(NQC):
                at_tiles = [
                    ps_at.tile([128, 2, QCHUNK], F32, tag="at", name=f"at{j}")
                    for j in range(2)
                ]

                def at_ap(pr):
                    return at_tiles[pr // 2][:, pr % 2, :]

                for kb in range(KB):
                    if qc == 0 and kb % 4 == 0 and kb // 4 + 1 < KB // 4:
                        c = kb // 4 + 1
                        proj_chunk(kT_d, KT_sb, wk_sb, c)
                        v_chunk(c)
                    if qc == 0 and kb == 2:
                        for cq in range(1, QS // 512):
                            proj_chunk(qT_d, QT_sb, wq_sb, cq)
                    mt = mpool.tile([128, QCHUNK], FP8E5, tag="mt", name="mt")
                    nc.sync.dma_start(
                        out=mt, in_=maskP_d[ts(kb, 128), ts(qc, QCHUNK)]
                    )
                    mrhs = bass.AP(
                        tensor=mt.tensor,
                        offset=mt.offset,
                        ap=[mt.ap[0], [0, 2], mt.ap[1]],
                    )
                    pbs = []
                    for wave in range(4):
                        # Wave-granular ACT/DVE split (act_of_8 eighths per
                        # 2-kb period) so both engines stay fed every kb.
                        e = (kb % 2) * 4 + wave
                        is_act = ((0, 1, 3, 4, 6, 2, 5, 7).index(e) < act_of_8)
                        half = wave // 2
                        sc = ps_sc.tile(
                            [128, 2, QCHUNK], F32, tag="sc", name="sc"
                        )
                        for i in range(2):
                            h = wave * 2 + i
                            strip = (h % 4) * 32
                            nc.tensor.matmul(
                                sc[:, i, :],
                                KT_sb[strip : strip + 32, half, ts(kb, 128)],
                                QT_sb[strip : strip + 32, half, ts(qc, QCHUNK)],
                                start=True,
                                stop=(not is_act),
                                tile_position=(strip, 0),
                            )
                        pb = ppool.tile([128, 2, QCHUNK], F16, tag="pb", name="pb")
                        if is_act:
                            for i in range(2):
                                nc.tensor.matmul(
                                    sc[:, i, :],
                                    identdr,
                                    mrhs,
                                    start=False,
                                    stop=True,
                                    perf_mode=mybir.MatmulPerfMode.DoubleRow,
                                )
                            nc.scalar.activation(
                                pb[:, :, :],
                                sc[:, :, :],
                                mybir.ActivationFunctionType.Exp,
                                scale=1.0 / A10R,
                                bias=bias_t[:, 0:1],
                            )
                        else:
                            nc.vector.scalar_tensor_tensor(
                                out=pb.bitcast(I16),
                                in0=sc[:, :, :],
                                scalar=BC,
                                in1=mrhs,
                                op0=mybir.AluOpType.add,
                                op1=mybir.AluOpType.mult,
                            )
                        pbs.append(pb)
                    for wave in range(4):
                        pb = pbs[wave]
                        h0 = wave * 2
                        nc.tensor.matmul(
                            at_ap(wave)[0 : DH + 1, :],
                            V_sb[:, kb, h0, :],
                            pb[:, 0, :],
                            start=(kb == 0),
                            stop=(kb == KB - 1),
                            tile_position=(0, 0),
                        )
                        nc.tensor.matmul(
                            at_ap(wave)[64 : 64 + DH + 1, :],
                            V_sb[:, kb, h0 + 1, :],
                            pb[:, 1, :],
                            start=(kb == 0),
                            stop=(kb == KB - 1),
                            tile_position=(0, 64),
                        )

                # normalize: attnT = attn_unnorm^T * (1/denom) broadcast
                for j in range(2):
                    for i in range(2):
                        base = i * 64
                        rec = rpool.tile([1, 2, QCHUNK], F32, tag="rc", name="rec")
                        nc.vector.reciprocal(
                            rec, at_tiles[j][base + DH : base + DH + 1, :, :]
                        )
                        for pr in range(2):
                            w = j * 2 + pr
                            h = w * 2 + i
                            rb = small.tile([DH, QCHUNK], F32, tag="rb", name="rb")
                            nc.gpsimd.partition_broadcast(rb, rec[0:1, pr, :])
                            nc.vector.tensor_mul(
                                attnT_sb[ts(h % 4, DH), h // 4, ts(qc, QCHUNK)],
                                at_tiles[j][base : base + DH, pr, :],
                                rb,
                            )

                # output projection for this q-chunk
                for qb in range(QCHUNK // 128):
                    qoff = qc * QCHUNK + qb * 128
                    po = ps_sc.tile([128, 2, 512], F32, tag="sc", name="po")
                    nc.tensor.matmul(
                        po[:, 0, 0:D],
                        attnT_sb[:, 0, qoff : qoff + 128],
                        wo_sb[:, 0, :],
                        start=True,
                        stop=False,
                    )
                    nc.tensor.matmul(
                        po[:, 0, 0:D],
                        attnT_sb[:, 1, qoff : qoff + 128],
                        wo_sb[:, 1, :],
                        start=False,
                        stop=True,
                    )
                    ot = ostage.tile([128, D], F32, tag="ot", name="ot")
                    nc.scalar.copy(ot, po[:, 0, 0:D])
                    nc.sync.dma_start(out=out_d[qoff : qoff + 128, :], in_=ot)

    nc.finalize()
    return nc


_NC_CACHE = None


def _get_nc():
    global _NC_CACHE
    if _NC_CACHE is None:
        _NC_CACHE = build_nc(int(os.environ.get("K_ACT_OF_8", str(ACT_OF_8))))
    return _NC_CACHE


def _prep_in_maps(q, k, v, mask, wq, wk, wv, w_out):
    f32 = np.float32
    bf16 = ml_dtypes.bfloat16
    qT = np.ascontiguousarray(np.transpose(np.asarray(q, f32), (0, 2, 1)))
    kT = np.ascontiguousarray(np.transpose(np.asarray(k, f32), (0, 2, 1)))
    vT = np.ascontiguousarray(np.transpose(np.asarray(v, f32), (0, 2, 1)))
    fp8e5 = ml_dtypes.float8_e5m2
    maskT = np.transpose(np.asarray(mask, bool), (0, 2, 1))
    maskP = (~maskT).astype(f32).astype(fp8e5)
    identdr = np.zeros((128, 2, 128), fp8e5)
    identdr[:, 0, :] = np.eye(128, dtype=f32) * f32(IDENT_DIAG)
    wq = np.asarray(wq, f32) * f32(A10)
    wk = np.asarray(wk, f32)
    wv = np.asarray(wv, f32)
    wo = np.asarray(w_out, f32)

    in_maps = []
    for c in range(NCORES):
        b = c // CORES_PER_B
        qs = slice((c % CORES_PER_B) * QS, (c % CORES_PER_B + 1) * QS)
        in_maps.append(
            {
                "qT": np.ascontiguousarray(qT[b][:, qs]),
                "kT": kT[b],
                "vT": vT[b],
                "maskP": np.ascontiguousarray(maskP[b][:, qs]),
                "wq": wq,
                "wk": wk,
                "wv": wv,
                "wo": wo,
                "identdr": identdr,
            }
        )
    return in_maps


def kernel(q, k, v, mask, wq, wk, wv, w_out):
    global LAST_EXEC_NS
    nc = _get_nc()
    in_maps = _prep_in_maps(q, k, v, mask, wq, wk, wv, w_out)
    trace = bool(os.environ.get("KERNEL_TRACE"))
    try:
        res = run_bass_kernel_spmd(nc, in_maps, list(range(NCORES)), trace=trace)
    except Exception:
        # A wedged NeuronCore (NRT_EXEC_UNIT_UNRECOVERABLE) is usually
        # transient under axon; one retry after a reset request recovers it.
        os.environ["NEURON_RT_RESET_CORES"] = "1"
        time.sleep(2)
        res = run_bass_kernel_spmd(nc, in_maps, list(range(NCORES)), trace=trace)
    LAST_EXEC_NS = res.exec_time_ns
    out = np.empty((B, S, D), np.float32)
    for c in range(NCORES):
        b = c // CORES_PER_B
        qs = slice((c % CORES_PER_B) * QS, (c % CORES_PER_B + 1) * QS)
        out[b, qs] = res.results[c]["out"]
    return out
